# revision 13
# baseline (speedup 1.0000x reference)
"""Trainium2 Bass kernel for nn_ODEFunc_interaction (gnn_message_passing).

Math (see reference):
  dz_dt = tanh([z, t] @ vW1 + vb1) @ vW2 + vb2                    (v-net, all rows)
  for each pair (perm[2i], perm[2i+1]):
      d_i  = z[perm[2i]] - z[perm[2i+1]]
      g_i  = grad_phi(d_i) = pW1 @ (pW2[:,0] * (1 - tanh(d_i@pW1 + pb1)^2))
      out[perm[2i]]   = dz_dt[perm[2i]]   - g_i
      out[perm[2i+1]] = dz_dt[perm[2i+1]] + g_i
  last 3 rows (triple) handled on host in float64 (tiny).

Strategy: host gathers z[perm] so each of 8 cores owns a contiguous block of
200000/8 = 25000 rows (12500 pairs). On-device layout is transposed+packed:
X[128, 6250] where partition 32*j+d holds dim d of row-chunk j (4 chunks of
6250 rows). All matmuls run as fp32r (full-rate fp32) on PE sub-tiles via
tile_position quadrants; tanh (+bias) on ACT; pair-diff and square on GPSIMD;
(1-u^2) and final +/- combine on DVE. Host scatters the result back by perm.
"""

import os
import numpy as np

B, D, H = 200003, 32, 128
NCORES = 8
P2 = 200000            # rows covered by pairs
RPC = P2 // NCORES     # 25000 rows per core
NCHUNK = 4
L = RPC // NCHUNK      # 6250 packed columns per core
LP = L + 2             # padded to keep every fp32r matmul free-size even
G = 1024               # column block (2 PSUM banks)

_CACHE = {}
LAST_RESULTS = None    # BassKernelResults of the most recent run (for test.py)


def build_program():
    """Build the single-core Bass/Tile program (same program runs SPMD on 8 cores)."""
    from contextlib import ExitStack
    import concourse.bacc as bacc
    import concourse.mybir as mybir
    import concourse.tile as tile

    dt = mybir.dt
    F32, F32R = dt.float32, dt.float32r
    AF = mybir.ActivationFunctionType
    OP = mybir.AluOpType

    # Everything feeding a matmul is typed float32r end-to-end (same bits as
    # f32 in memory; walrus's verifier requires producers typed as f32r).
    nc = bacc.Bacc()
    X = nc.dram_tensor("x", [128, LP], F32R, kind="ExternalInput")
    W1 = nc.dram_tensor("w1rep", [128, H], F32R, kind="ExternalInput")
    BH = nc.dram_tensor("biash", [128, 1], F32, kind="ExternalInput")
    PW1 = nc.dram_tensor("pw1rep", [128, H], F32R, kind="ExternalInput")
    PB1 = nc.dram_tensor("pb1c", [128, 1], F32, kind="ExternalInput")
    # matmul outputs must start at PSUM partition 0, so the second-layer
    # weights are column-placed per chunk (vW2 at columns 32j, zeros
    # elsewhere); the 4 chunk matmuls accumulate into one [128,*] psum tile.
    W2Q = nc.dram_tensor("w2q", [128, 4 * H], F32R, kind="ExternalInput")
    PWTQ = nc.dram_tensor("pwtq", [128, 4 * H], F32R, kind="ExternalInput")
    # zero-padded first-layer variants for chunk 3 (partition base 96 is not
    # encodable: access from base 64 with K=64; rows 64:96 are zeros)
    W1Z = nc.dram_tensor("w1z", [128, H], F32R, kind="ExternalInput")
    PW1Z = nc.dram_tensor("pw1z", [128, H], F32R, kind="ExternalInput")
    O = nc.dram_tensor("out", [128, LP], F32, kind="ExternalOutput")

    with tile.TileContext(nc) as tc, ExitStack() as ctx:
        wpool = ctx.enter_context(tc.tile_pool(name="wpool", bufs=1))
        xpool = ctx.enter_context(tc.tile_pool(name="xpool", bufs=3))
        upool = ctx.enter_context(tc.tile_pool(name="upool", bufs=2))
        vpool = ctx.enter_context(tc.tile_pool(name="vpool", bufs=2))
        sqpool = ctx.enter_context(tc.tile_pool(name="sqpool", bufs=2))
        stpool = ctx.enter_context(tc.tile_pool(name="stpool", bufs=2))
        dpool = ctx.enter_context(tc.tile_pool(name="dpool", bufs=2))
        gspool = ctx.enter_context(tc.tile_pool(name="gspool", bufs=2))
        opool = ctx.enter_context(tc.tile_pool(name="opool", bufs=3))
        hps = ctx.enter_context(tc.tile_pool(name="hps", bufs=2, space="PSUM"))
        dzps = ctx.enter_context(tc.tile_pool(name="dzps", bufs=1, space="PSUM"))
        aps = ctx.enter_context(tc.tile_pool(name="aps", bufs=1, space="PSUM"))
        gps = ctx.enter_context(tc.tile_pool(name="gps", bufs=1, space="PSUM"))

        w1 = wpool.tile([128, H], F32R)
        nc.sync.dma_start(w1[:], W1[:])
        bh = wpool.tile([128, 1], F32)
        nc.sync.dma_start(bh[:], BH[:])
        pw1 = wpool.tile([128, H], F32R)
        nc.sync.dma_start(pw1[:], PW1[:])
        pb1 = wpool.tile([128, 1], F32)
        nc.sync.dma_start(pb1[:], PB1[:])
        w2q = wpool.tile([128, 4 * H], F32R)
        nc.sync.dma_start(w2q[:], W2Q[:])
        pwtq = wpool.tile([128, 4 * H], F32R)
        nc.sync.dma_start(pwtq[:], PWTQ[:])
        w1z = wpool.tile([128, H], F32R)
        nc.sync.dma_start(w1z[:], W1Z[:])
        pw1z = wpool.tile([128, H], F32R)
        nc.sync.dma_start(pw1z[:], PW1Z[:])

        for c0 in range(0, LP, G):
            W_ = min(G, LP - c0)
            Wp = W_ // 2
            xt = xpool.tile([128, G], F32R)
            nc.sync.dma_start(xt[:, :W_], X[:, c0 : c0 + W_])

            df = dpool.tile([128, G // 2], F32R)
            nc.gpsimd.tensor_tensor(df[:, :Wp], xt[:, 0:W_:2], xt[:, 1:W_:2], OP.subtract)

            dz = dzps.tile([128, G], F32)
            gp = gps.tile([128, G // 2], F32)
            ot = opool.tile([128, G], F32)

            # j=3 first: its M=64 writes (start=True) clear psum partitions
            # 64:96 to zero; j=2 then accumulates its strip on top (start=False).
            for j in (3, 0, 1, 2):
                p0 = 32 * j
                ph = hps.tile([128, G], F32)
                for s0 in range(0, W_, 512):
                    sw = min(512, W_ - s0)
                    if j == 3:
                        nc.tensor.matmul(
                            ph[:, s0 : s0 + sw],
                            w1z[64:128, :],
                            xt[64:128, s0 : s0 + sw],
                            start=True,
                            stop=True,
                        )
                    else:
                        nc.tensor.matmul(
                            ph[:, s0 : s0 + sw],
                            w1[p0 : p0 + 32, :],
                            xt[p0 : p0 + 32, s0 : s0 + sw],
                            start=True,
                            stop=True,
                        )
                ut = upool.tile([128, G], F32R)
                nc.scalar.activation(ut[:, :W_], ph[:, :W_], AF.Tanh, bias=bh[:])
                for s0 in range(0, W_, 512):
                    sw = min(512, W_ - s0)
                    nc.tensor.matmul(
                        dz[:, s0 : s0 + sw],
                        w2q[:, H * j : H * (j + 1)],
                        ut[:, s0 : s0 + sw],
                        start=(j == 3),
                        stop=(j == 2),
                        skip_group_check=True,
                    )
                pa = aps.tile([128, G // 2], F32)
                if j == 3:
                    nc.tensor.matmul(
                        pa[:, :Wp],
                        pw1z[64:128, :],
                        df[64:128, :Wp],
                        start=True,
                        stop=True,
                    )
                else:
                    nc.tensor.matmul(
                        pa[:, :Wp],
                        pw1[p0 : p0 + 32, :],
                        df[p0 : p0 + 32, :Wp],
                        start=True,
                        stop=True,
                    )
                vt = vpool.tile([128, G // 2], F32)
                nc.scalar.activation(vt[:, :Wp], pa[:, :Wp], AF.Tanh, bias=pb1[:])
                sq = sqpool.tile([128, G // 2], F32)
                nc.gpsimd.tensor_mul(sq[:, :Wp], vt[:, :Wp], vt[:, :Wp])
                st = stpool.tile([128, G // 2], F32R)
                nc.vector.tensor_scalar(st[:, :Wp], sq[:, :Wp], -1.0, 1.0, OP.mult, OP.add)
                nc.tensor.matmul(
                    gp[:, :Wp],
                    pwtq[:, H * j : H * (j + 1)],
                    st[:, :Wp],
                    start=(j == 3),
                    stop=(j == 2),
                    skip_group_check=True,
                )

            gs = gspool.tile([128, G // 2], F32)
            nc.vector.tensor_copy(gs[:, :Wp], gp[:, :Wp])
            nc.vector.tensor_tensor(ot[:, 0:W_:2], dz[:, 0:W_:2], gs[:, :Wp], OP.subtract)
            nc.vector.tensor_tensor(ot[:, 1:W_:2], dz[:, 1:W_:2], gs[:, :Wp], OP.add)
            nc.sync.dma_start(O[:, c0 : c0 + W_], ot[:, :W_])

    nc.compile()
    return nc


def _prep_weights(t, vW1, vb1, vW2, vb2, pW1, pb1, pW2):
    f32 = np.float32
    t = np.asarray(t, dtype=f32).reshape(-1)[0]
    vW1 = np.asarray(vW1, dtype=f32)
    w1rep = np.tile(np.ascontiguousarray(vW1[:32]), (4, 1))            # [128,128]
    biash = (np.asarray(vb1, f32) + t * vW1[32]).reshape(128, 1).astype(f32)
    vw2 = np.ascontiguousarray(np.asarray(vW2, f32))                   # [128,32]
    pW1 = np.asarray(pW1, f32)
    pw1rep = np.tile(pW1, (4, 1))                                      # [128,128]
    pb1c = np.asarray(pb1, f32).reshape(128, 1).copy()
    w2col = np.asarray(pW2, f32).reshape(128)
    pw1tw2 = np.ascontiguousarray((pW1 * w2col[None, :]).T)            # [128,32]
    z96 = np.zeros((96, 128), f32)
    w2q = np.zeros((128, 512), f32)
    pwtq = np.zeros((128, 512), f32)
    for j in range(4):
        w2q[:, 128 * j + 32 * j : 128 * j + 32 * j + 32] = vw2
        pwtq[:, 128 * j + 32 * j : 128 * j + 32 * j + 32] = pw1tw2
    return {
        "w1rep": np.ascontiguousarray(w1rep),
        "biash": biash,
        "pw1rep": np.ascontiguousarray(pw1rep),
        "pb1c": pb1c,
        "w2q": w2q,
        "pwtq": pwtq,
        "w1z": np.ascontiguousarray(np.vstack([z96, vW1[:32]])),       # [128,128]
        "pw1z": np.ascontiguousarray(np.vstack([z96, pW1])),           # [128,128]
    }


def _pack_core(zc):
    """[25000, 32] f32 -> [128, 6252] packed (partition 32*j+d, col i = row j*L+i),
    padded with 2 zero columns."""
    out = np.zeros((128, LP), dtype=np.float32)
    out[:, :L] = zc.reshape(NCHUNK, L, 32).transpose(0, 2, 1).reshape(128, L)
    return out


def _unpack_core(oc):
    """[128, 6252] packed -> [25000, 32]."""
    return oc[:, :L].reshape(NCHUNK, 32, L).transpose(0, 2, 1).reshape(RPC, 32)


def _host_triple(t, z3, vW1, vb1, vW2, vb2, pW1, pb1, pW2):
    """Exact float64 computation of the 3 leftover rows: dz_dt + triple forces."""
    f8 = np.float64
    z3 = z3.astype(f8)
    vW1 = np.asarray(vW1, f8)
    t = float(np.asarray(t).reshape(-1)[0])
    h3 = np.tanh(z3 @ vW1[:32] + t * vW1[32] + np.asarray(vb1, f8))
    dz3 = h3 @ np.asarray(vW2, f8) + np.asarray(vb2, f8)

    pW1 = np.asarray(pW1, f8)
    w2 = np.asarray(pW2, f8).reshape(128)
    d9 = (z3[:, None, :] - z3[None, :, :]).reshape(9, 32)
    u9 = np.tanh(d9 @ pW1 + np.asarray(pb1, f8))
    s9 = (1.0 - u9 * u9) * w2[None, :]
    g9 = s9 @ pW1.T                       # grad_phi rows
    f9 = (-g9).reshape(3, 3, 32)
    f9 = f9 * (1.0 - np.eye(3)[:, :, None])
    force3 = f9.sum(axis=1) * 2.0
    return (dz3 + force3).astype(np.float32)


def kernel(t, z, perm, vW1, vb1, vW2, vb2, pW1, pb1, pW2, pb2):
    from concourse.bass_utils import run_bass_kernel_spmd

    global LAST_RESULTS
    if "nc" not in _CACHE:
        _CACHE["nc"] = build_program()
    nc = _CACHE["nc"]

    z = np.asarray(z, np.float32)
    perm = np.asarray(perm)
    weights = _prep_weights(t, vW1, vb1, vW2, vb2, pW1, pb1, pW2)

    zg = z[perm[:P2]]                       # [200000, 32] gathered pair rows
    in_maps = []
    for c in range(NCORES):
        im = {"x": _pack_core(zg[c * RPC : (c + 1) * RPC])}
        im.update(weights)
        in_maps.append(im)

    trace = bool(int(os.environ.get("KERNEL_TRACE", "0")))
    res = run_bass_kernel_spmd(nc, in_maps, list(range(NCORES)), trace=trace)
    LAST_RESULTS = res

    out = np.empty((B, 32), dtype=np.float32)
    og = np.concatenate([_unpack_core(res.results[c]["out"]) for c in range(NCORES)], axis=0)
    og += np.asarray(vb2, np.float32)[None, :]
    out[perm[:P2]] = og
    out[perm[P2:]] = _host_triple(t, z[perm[P2:]], vW1, vb1, vW2, vb2, pW1, pb1, pW2)
    return out


# revision 15
# speedup vs baseline: 1.3416x; 1.3416x over previous
"""Trainium2 Bass kernel for nn_ODEFunc_interaction (gnn_message_passing).

Math (see reference):
  dz_dt = tanh([z, t] @ vW1 + vb1) @ vW2 + vb2                    (v-net, all rows)
  for each pair (perm[2i], perm[2i+1]):
      d_i  = z[perm[2i]] - z[perm[2i+1]]
      g_i  = grad_phi(d_i) = pW1 @ (pW2[:,0] * (1 - tanh(d_i@pW1 + pb1)^2))
      out[perm[2i]]   = dz_dt[perm[2i]]   - g_i
      out[perm[2i+1]] = dz_dt[perm[2i+1]] + g_i
  last 3 rows (triple) handled on host in float64 (tiny).

Strategy: host gathers z[perm] so each of 8 cores owns a contiguous block of
200000/8 = 25000 rows (12500 pairs). On-device layout is transposed+packed:
X[128, 6250] where partition 32*j+d holds dim d of row-chunk j (4 chunks of
6250 rows). All matmuls run as fp32r (full-rate fp32) on PE sub-tiles via
tile_position quadrants; tanh (+bias) on ACT; pair-diff and square on GPSIMD;
(1-u^2) and final +/- combine on DVE. Host scatters the result back by perm.
"""

import os
import numpy as np

B, D, H = 200003, 32, 128
NCORES = 8
P2 = 200000            # rows covered by pairs
RPC = P2 // NCORES     # 25000 rows per core
NCHUNK = 4
L = RPC // NCHUNK      # 6250 packed columns per core
LP = L + 2             # padded to keep every fp32r matmul free-size even
G = 1024               # column block (2 PSUM banks)

_CACHE = {}
LAST_RESULTS = None    # BassKernelResults of the most recent run (for test.py)


def build_program():
    """Build the single-core Bass/Tile program (same program runs SPMD on 8 cores)."""
    from contextlib import ExitStack
    import concourse.bacc as bacc
    import concourse.mybir as mybir
    import concourse.tile as tile

    dt = mybir.dt
    F32, F32R = dt.float32, dt.float32r
    AF = mybir.ActivationFunctionType
    OP = mybir.AluOpType

    F16 = dt.float16
    # All matmul streams run in fp16 (fp32r measured ~3 cyc/col on HW; fp16
    # streams at 1 cyc/col and halves the input DMA). Accuracy ~4e-4 rel.
    # One concatenated fp16 weight tensor [128, 1536]:
    #   w1rep[0:128] | pw1rep[128:256] | w2q[256:768] | pwtq[768:1280]
    #   | w1z[1280:1408] | pw1z[1408:1536]
    # w2q/pwtq are column-placed per chunk (vW2 at columns 32j of block j,
    # zeros elsewhere): matmul outputs must start at PSUM partition 0, so the
    # 4 chunk matmuls accumulate full-M into one [128,*] psum tile.
    # w1z/pw1z: chunk 3 is read from partition base 64 with K=64 and zeros in
    # rows 64:96 (partition base 96 is not encodable).
    nc = bacc.Bacc()
    X = nc.dram_tensor("x", [128, LP], F16, kind="ExternalInput")
    WC = nc.dram_tensor("wcat", [128, 1536], F16, kind="ExternalInput")
    BC = nc.dram_tensor("bias", [128, 2], F32, kind="ExternalInput")
    O = nc.dram_tensor("out", [128, LP], F32, kind="ExternalOutput")

    with tile.TileContext(nc) as tc, ExitStack() as ctx:
        wpool = ctx.enter_context(tc.tile_pool(name="wpool", bufs=1))
        xpool = ctx.enter_context(tc.tile_pool(name="xpool", bufs=3))
        upool = ctx.enter_context(tc.tile_pool(name="upool", bufs=2))
        vpool = ctx.enter_context(tc.tile_pool(name="vpool", bufs=2))
        sqpool = ctx.enter_context(tc.tile_pool(name="sqpool", bufs=2))
        stpool = ctx.enter_context(tc.tile_pool(name="stpool", bufs=2))
        dpool = ctx.enter_context(tc.tile_pool(name="dpool", bufs=2))
        gspool = ctx.enter_context(tc.tile_pool(name="gspool", bufs=2))
        opool = ctx.enter_context(tc.tile_pool(name="opool", bufs=3))
        hps = ctx.enter_context(tc.tile_pool(name="hps", bufs=2, space="PSUM"))
        dzps = ctx.enter_context(tc.tile_pool(name="dzps", bufs=1, space="PSUM"))
        aps = ctx.enter_context(tc.tile_pool(name="aps", bufs=1, space="PSUM"))
        gps = ctx.enter_context(tc.tile_pool(name="gps", bufs=1, space="PSUM"))

        wt = wpool.tile([128, 1536], F16)
        nc.sync.dma_start(wt[:], WC[:])
        bt = wpool.tile([128, 2], F32)
        nc.sync.dma_start(bt[:], BC[:])
        w1 = wt[:, 0:128]
        pw1 = wt[:, 128:256]
        w2q = wt[:, 256:768]
        pwtq = wt[:, 768:1280]
        w1z = wt[:, 1280:1408]
        pw1z = wt[:, 1408:1536]
        bh = bt[:, 0:1]
        pb1 = bt[:, 1:2]

        for c0 in range(0, LP, G):
            W_ = min(G, LP - c0)
            Wp = W_ // 2
            xt = xpool.tile([128, G], F16)
            nc.sync.dma_start(xt[:, :W_], X[:, c0 : c0 + W_])

            df = dpool.tile([128, G // 2], F16)
            nc.gpsimd.tensor_tensor(df[:, :Wp], xt[:, 0:W_:2], xt[:, 1:W_:2], OP.subtract)

            dz = dzps.tile([128, G], F32)
            gp = gps.tile([128, G // 2], F32)
            ot = opool.tile([128, G], F32)

            # j=3 first: its M=64 writes (start=True) clear psum partitions
            # 64:96 to zero; j=2 then accumulates its strip on top (start=False).
            for j in (3, 0, 1, 2):
                p0 = 32 * j
                ph = hps.tile([128, G], F32)
                for s0 in range(0, W_, 512):
                    sw = min(512, W_ - s0)
                    if j == 3:
                        nc.tensor.matmul(
                            ph[:, s0 : s0 + sw],
                            w1z[64:128],
                            xt[64:128, s0 : s0 + sw],
                            start=True,
                            stop=True,
                        )
                    else:
                        nc.tensor.matmul(
                            ph[:, s0 : s0 + sw],
                            w1[p0 : p0 + 32, :],
                            xt[p0 : p0 + 32, s0 : s0 + sw],
                            start=True,
                            stop=True,
                        )
                ut = upool.tile([128, G], F16)
                nc.scalar.activation(ut[:, :W_], ph[:, :W_], AF.Tanh, bias=bh[:])
                for s0 in range(0, W_, 512):
                    sw = min(512, W_ - s0)
                    nc.tensor.matmul(
                        dz[:, s0 : s0 + sw],
                        w2q[:, H * j : H * (j + 1)],
                        ut[:, s0 : s0 + sw],
                        start=(j == 3),
                        stop=(j == 2),
                        skip_group_check=True,
                    )
                pa = aps.tile([128, G // 2], F32)
                if j == 3:
                    nc.tensor.matmul(
                        pa[:, :Wp],
                        pw1z[64:128],
                        df[64:128, :Wp],
                        start=True,
                        stop=True,
                    )
                else:
                    nc.tensor.matmul(
                        pa[:, :Wp],
                        pw1[p0 : p0 + 32, :],
                        df[p0 : p0 + 32, :Wp],
                        start=True,
                        stop=True,
                    )
                vt = vpool.tile([128, G // 2], F16)
                nc.scalar.activation(vt[:, :Wp], pa[:, :Wp], AF.Tanh, bias=pb1[:])
                sq = sqpool.tile([128, G // 2], F16)
                nc.gpsimd.tensor_mul(sq[:, :Wp], vt[:, :Wp], vt[:, :Wp])
                st = stpool.tile([128, G // 2], F16)
                nc.vector.tensor_scalar(st[:, :Wp], sq[:, :Wp], -1.0, 1.0, OP.mult, OP.add)
                nc.tensor.matmul(
                    gp[:, :Wp],
                    pwtq[:, H * j : H * (j + 1)],
                    st[:, :Wp],
                    start=(j == 3),
                    stop=(j == 2),
                    skip_group_check=True,
                )

            gs = gspool.tile([128, G // 2], F32)
            nc.vector.tensor_copy(gs[:, :Wp], gp[:, :Wp])
            nc.vector.tensor_tensor(ot[:, 0:W_:2], dz[:, 0:W_:2], gs[:, :Wp], OP.subtract)
            nc.vector.tensor_tensor(ot[:, 1:W_:2], dz[:, 1:W_:2], gs[:, :Wp], OP.add)
            nc.sync.dma_start(O[:, c0 : c0 + W_], ot[:, :W_])

    nc.compile()
    return nc


def _prep_weights(t, vW1, vb1, vW2, vb2, pW1, pb1, pW2):
    f32 = np.float32
    t = np.asarray(t, dtype=f32).reshape(-1)[0]
    vW1 = np.asarray(vW1, dtype=f32)
    w1rep = np.tile(np.ascontiguousarray(vW1[:32]), (4, 1))            # [128,128]
    biash = (np.asarray(vb1, f32) + t * vW1[32]).reshape(128, 1).astype(f32)
    vw2 = np.ascontiguousarray(np.asarray(vW2, f32))                   # [128,32]
    pW1 = np.asarray(pW1, f32)
    pw1rep = np.tile(pW1, (4, 1))                                      # [128,128]
    pb1c = np.asarray(pb1, f32).reshape(128, 1).copy()
    w2col = np.asarray(pW2, f32).reshape(128)
    pw1tw2 = np.ascontiguousarray((pW1 * w2col[None, :]).T)            # [128,32]
    z96 = np.zeros((96, 128), f32)
    w2q = np.zeros((128, 512), f32)
    pwtq = np.zeros((128, 512), f32)
    for j in range(4):
        w2q[:, 128 * j + 32 * j : 128 * j + 32 * j + 32] = vw2
        pwtq[:, 128 * j + 32 * j : 128 * j + 32 * j + 32] = pw1tw2
    w1z = np.vstack([z96, vW1[:32]])                                   # [128,128]
    pw1z = np.vstack([z96, pW1])                                       # [128,128]
    wcat = np.hstack([w1rep, pw1rep, w2q, pwtq, w1z, pw1z]).astype(np.float16)
    bias = np.hstack([biash, pb1c]).astype(f32)
    return {"wcat": np.ascontiguousarray(wcat), "bias": np.ascontiguousarray(bias)}


def _pack_core(zc):
    """[25000, 32] f32 -> [128, 6252] fp16 packed (partition 32*j+d, col i =
    row j*L+i), padded with 2 zero columns."""
    out = np.zeros((128, LP), dtype=np.float16)
    out[:, :L] = zc.reshape(NCHUNK, L, 32).transpose(0, 2, 1).reshape(128, L)
    return out


def _unpack_core(oc):
    """[128, 6252] packed -> [25000, 32]."""
    return oc[:, :L].reshape(NCHUNK, 32, L).transpose(0, 2, 1).reshape(RPC, 32)


def _host_triple(t, z3, vW1, vb1, vW2, vb2, pW1, pb1, pW2):
    """Exact float64 computation of the 3 leftover rows: dz_dt + triple forces."""
    f8 = np.float64
    z3 = z3.astype(f8)
    vW1 = np.asarray(vW1, f8)
    t = float(np.asarray(t).reshape(-1)[0])
    h3 = np.tanh(z3 @ vW1[:32] + t * vW1[32] + np.asarray(vb1, f8))
    dz3 = h3 @ np.asarray(vW2, f8) + np.asarray(vb2, f8)

    pW1 = np.asarray(pW1, f8)
    w2 = np.asarray(pW2, f8).reshape(128)
    d9 = (z3[:, None, :] - z3[None, :, :]).reshape(9, 32)
    u9 = np.tanh(d9 @ pW1 + np.asarray(pb1, f8))
    s9 = (1.0 - u9 * u9) * w2[None, :]
    g9 = s9 @ pW1.T                       # grad_phi rows
    f9 = (-g9).reshape(3, 3, 32)
    f9 = f9 * (1.0 - np.eye(3)[:, :, None])
    force3 = f9.sum(axis=1) * 2.0
    return (dz3 + force3).astype(np.float32)


def kernel(t, z, perm, vW1, vb1, vW2, vb2, pW1, pb1, pW2, pb2):
    from concourse.bass_utils import run_bass_kernel_spmd

    global LAST_RESULTS
    if "nc" not in _CACHE:
        _CACHE["nc"] = build_program()
    nc = _CACHE["nc"]

    z = np.asarray(z, np.float32)
    perm = np.asarray(perm)
    weights = _prep_weights(t, vW1, vb1, vW2, vb2, pW1, pb1, pW2)

    zg = z[perm[:P2]]                       # [200000, 32] gathered pair rows
    in_maps = []
    for c in range(NCORES):
        im = {"x": _pack_core(zg[c * RPC : (c + 1) * RPC])}
        im.update(weights)
        in_maps.append(im)

    trace = bool(int(os.environ.get("KERNEL_TRACE", "0")))
    res = run_bass_kernel_spmd(nc, in_maps, list(range(NCORES)), trace=trace)
    LAST_RESULTS = res

    out = np.empty((B, 32), dtype=np.float32)
    og = np.concatenate([_unpack_core(res.results[c]["out"]) for c in range(NCORES)], axis=0)
    og += np.asarray(vb2, np.float32)[None, :]
    out[perm[:P2]] = og
    out[perm[P2:]] = _host_triple(t, z[perm[P2:]], vW1, vb1, vW2, vb2, pW1, pb1, pW2)
    return out
